# revision 2
# baseline (speedup 1.0000x reference)
"""MoE update-MLP Trainium2 kernel v4: host-routed top-2 sparsity, 8-core SPMD,
software-pipelined across tiles.

Like v3 (host evaluates the router with the reference's own jax fp32 einsum,
groups the 131072 active (pixel, expert) units by expert into 512-slot tiles,
cores each run NT fixed tile slots, output unsharded host-side by summing each
pixel's two slot contributions), with two throughput fixes:

1. Tile-level software pipeline. v3's per-tile chain L1 -> gelu -> L2 -> gelu
   -> gate-mult -> L3 stalled the PE ~35% of the time waiting on ACT/DVE,
   which also dropped the PE out of its ramped p-state (half-rate ~50% of the
   run). v4 emits per iteration: A(t) = gate-broadcast + L1 matmuls,
   C(t-1) = L3 + b3 + output copy, B(t) = L2 matmuls + gelu + gate-mult, so
   every ACT/DVE stage runs under another tile's matmuls.

2. Tiles are paired by expert (each expert's tile count padded to even, pairs
   dealt round-robin to cores), so one weight-block DMA serves two tiles,
   halving the dominant DMA stream.

Device math is fp32r (full PE rate at free-dim 512, and unlike bf16/fp8 it
avoids the heavy power-throttle observed on dense 16/8-bit streams); PSUM is
fp32; gates stay fp32 end-to-end. L1's bias rides a ones-row in the padded x
chunk; L2 uses per-m ACT bias; b3 enters as a K=1 matmul of the b3 row
against the gate row.
"""

import numpy as np

import concourse.bacc as bacc
import concourse.mybir as mybir
import concourse.tile as tile
from concourse.bass import broadcast_tensor_aps
from concourse.bass_utils import run_bass_kernel_spmd

F32 = mybir.dt.float32
F32R = mybir.dt.float32r
AF = mybir.ActivationFunctionType
ALU = mybir.AluOpType

N_CORES = 8
B, IN_C, H, W = 4, 192, 128, 128
R_C, E, HID, OUT_C = 8, 8, 384, 192
TILE = 512
NP_ = 17                      # weight pairs per core
NT = 2 * NP_                  # tile slots per core
NPIX = B * H * W

W1_OFF = 0
W2_OFF = 2 * HID
W3_OFF = 2 * HID + 3 * HID
WBLK = 2 * HID + 3 * HID + 3 * OUT_C

_nc_cache: dict = {}


def _build(compile: bool = True):
    nc = bacc.Bacc("TRN2", target_bir_lowering=False, debug=False)

    xt_in = nc.declare_dram_parameter("xt", [NT, 128, 2, TILE], F32R, isOutput=False)
    wb_in = nc.declare_dram_parameter("wb", [NP_, 128, WBLK], F32R, isOutput=False)
    gt_in = nc.declare_dram_parameter("gt", [NT, 1, TILE + OUT_C], F32R, isOutput=False)
    b2_in = nc.declare_dram_parameter("b2t", [NP_, 128, 3], F32, isOutput=False)
    ones_in = nc.declare_dram_parameter("ones", [1, 128], F32R, isOutput=False)
    out_d = nc.declare_dram_parameter("out", [NT, OUT_C, TILE], F32, isOutput=True)

    with tile.TileContext(nc) as tc:
        with (
            tc.tile_pool(name="cpool", bufs=1) as cpool,
            tc.tile_pool(name="wpool", bufs=2) as wpool,
            tc.tile_pool(name="xpool", bufs=2) as xpool,
            tc.tile_pool(name="hpool", bufs=2) as hpool,
            tc.tile_pool(name="psL1", bufs=2, space="PSUM") as psL1,
            tc.tile_pool(name="psL2", bufs=1, space="PSUM") as psL2,
            tc.tile_pool(name="psG", bufs=3, space="PSUM") as psG,
        ):
            ones_sb = cpool.tile([1, 128], F32R)
            nc.sync.dma_start(ones_sb[:], ones_in[:])

            w_sb = {}
            x_sb = {}
            g_sb = {}
            b2_sb = {}
            h2g = {}
            o_ps = {}
            o_rd = {}

            def dma_tile(t):
                if t >= NT:
                    return
                if t % 2 == 0:
                    p = t // 2
                    w_sb[p] = wpool.tile([128, WBLK], F32R, tag="w", name=f"w_{p}")
                    nc.gpsimd.dma_start(w_sb[p][:], wb_in[p])
                    b2_sb[p] = xpool.tile([128, 3], F32, tag="b2", name=f"b2_{p}")
                    nc.sync.dma_start(b2_sb[p][:], b2_in[p])
                x_sb[t] = xpool.tile([128, 2, TILE], F32R, tag="x", name=f"x_{t}")
                nc.sync.dma_start(x_sb[t][:], xt_in[t])
                g_sb[t] = xpool.tile(
                    [1, TILE + OUT_C], F32R, tag="g", bufs=3, name=f"g_{t}"
                )
                nc.sync.dma_start(g_sb[t][:], gt_in[t])

            def emit_A(t):
                w = w_sb[t // 2]
                gb_ps = psG.tile([128, TILE], F32, tag="ps_g", name=f"gbps_{t}")
                nc.tensor.matmul(
                    gb_ps[:], ones_sb[:], g_sb[t][:, 0:TILE], start=True, stop=True
                )
                gb = hpool.tile([128, 1, TILE], F32R, tag="gb", name=f"gb_{t}")
                nc.vector.tensor_copy(gb[:, 0, :], gb_ps[:])
                h1 = hpool.tile([128, 3, TILE], F32R, tag="h1", name=f"h1_{t}")
                for m in range(3):
                    ps1 = psL1.tile([128, TILE], F32, tag="ps1", name=f"ps1_{t}_{m}")
                    for c in range(2):
                        col = W1_OFF + c * HID + 128 * m
                        nc.tensor.matmul(
                            ps1[:],
                            w[:, col : col + 128],
                            x_sb[t][:, c, :],
                            start=(c == 0),
                            stop=(c == 1),
                        )
                    nc.scalar.activation(h1[:, m, :], ps1[:], AF.Gelu)
                return h1, gb

            def emit_B(t, h1, gb):
                w = w_sb[t // 2]
                ps2 = psL2.tile([128, 3, TILE], F32, tag="ps2", name=f"ps2_{t}")
                for m in range(3):
                    for k in range(3):
                        col = W2_OFF + k * HID + 128 * m
                        nc.tensor.matmul(
                            ps2[:, m, :],
                            w[:, col : col + 128],
                            h1[:, k, :],
                            start=(k == 0),
                            stop=(k == 2),
                        )
                h2 = hpool.tile([128, 3, TILE], F32R, tag="h2", name=f"h2_{t}")
                for m in range(3):
                    nc.scalar.activation(
                        h2[:, m, :], ps2[:, m, :], AF.Gelu,
                        bias=b2_sb[t // 2][:, m : m + 1],
                    )
                h2g[t] = hpool.tile([128, 3, TILE], F32R, tag="h2g", name=f"h2g_{t}")
                nc.vector.tensor_tensor(
                    h2g[t][:], *broadcast_tensor_aps(h2[:], gb[:]), op=ALU.mult
                )

            def emit_C(t):
                w = w_sb[t // 2]
                o_ps[t] = [
                    psG.tile([128, TILE], F32, tag="ps_g", name=f"o0_{t}"),
                    psG.tile([128, TILE], F32, tag="ps_g", name=f"o1_{t}"),
                ]
                for m, rows in ((0, 128), (1, OUT_C - 128)):
                    op = o_ps[t][m]
                    for k in range(3):
                        col = W3_OFF + k * OUT_C + 128 * m
                        nc.tensor.matmul(
                            op[:rows],
                            w[:, col : col + rows],
                            h2g[t][:, k, :],
                            start=(k == 0),
                            stop=False,
                        )
                    nc.tensor.matmul(
                        op[:rows],
                        g_sb[t][:, TILE + 128 * m : TILE + 128 * m + rows],
                        g_sb[t][:, 0:TILE],
                        start=False,
                        stop=True,
                    )

            def emit_C_read(t):
                for m, rows in ((0, 128), (1, OUT_C - 128)):
                    o_sb = hpool.tile([128, TILE], F32, tag="o_sb", bufs=4)
                    nc.vector.tensor_copy(o_sb[:rows], o_ps[t][m][:rows])
                    nc.sync.dma_start(
                        out_d[t, 128 * m : 128 * m + rows, :], o_sb[:rows]
                    )

            dma_tile(0)
            prev = None
            for t in range(NT):
                dma_tile(t + 1)
                h1, gb = emit_A(t)
                if prev is not None:
                    emit_C(prev)
                    emit_C_read(prev)
                emit_B(t, h1, gb)
                prev = t
            emit_C(prev)
            emit_C_read(prev)

    if compile:
        nc.compile()
    return nc


def _get_nc():
    if "v4" not in _nc_cache:
        _nc_cache["v4"] = _build()
    return _nc_cache["v4"]


def _route(router_input, router_W, router_b):
    """Host router. Logits via the same jax fp32 einsum as the reference so
    the top-2 ranking matches bit-for-bit; gates in fp64 from those logits."""
    import jax
    import jax.numpy as jnp

    cpu = jax.devices("cpu")[0]
    with jax.default_device(cpu):
        logits = np.asarray(
            jnp.einsum(
                "ec,bchw->behw",
                jnp.asarray(router_W, jnp.float32),
                jnp.asarray(router_input, jnp.float32),
            )
            + jnp.asarray(router_b, jnp.float32)[None, :, None, None]
        )
    lt = logits.transpose(0, 2, 3, 1).reshape(-1, E).astype(np.float64)
    top2 = np.argsort(-lt, axis=-1, kind="stable")[:, :2]
    l1 = np.take_along_axis(lt, top2[:, 0:1], 1)[:, 0]
    l2 = np.take_along_axis(lt, top2[:, 1:2], 1)[:, 0]
    e2 = np.exp(l2 - l1)
    g1 = 1.0 / (1.0 + e2)
    g2 = e2 / (1.0 + e2)
    return top2, np.stack([g1, g2], 1).astype(np.float32)


def make_in_maps(x, router_input, router_W, router_b, W1, b1, W2, b2, W3, b3):
    f = np.float32

    top2, gg = _route(router_input, router_W, router_b)

    ue = top2.reshape(-1)
    upix = np.repeat(np.arange(NPIX), 2)
    ug = gg.reshape(-1)
    order = np.argsort(ue, kind="stable")
    counts = np.bincount(ue, minlength=E)

    # pad each expert's tile count to EVEN so pairs share one weight block
    seg_tiles = [int(2 * np.ceil(np.ceil(c / TILE) / 2)) for c in counts]
    n_tiles = sum(seg_tiles)
    n_pairs = n_tiles // 2
    assert n_pairs <= NP_ * N_CORES, (counts, n_tiles)
    S = n_tiles * TILE
    slot_of_sorted = np.zeros(2 * NPIX, np.int64)
    pos = 0
    off = 0
    tile_e = []
    for e in range(E):
        c = int(counts[e])
        slot_of_sorted[off : off + c] = pos + np.arange(c)
        tile_e += [e] * seg_tiles[e]
        pos += seg_tiles[e] * TILE
        off += c
    slot_of_unit = np.zeros(2 * NPIX, np.int64)
    slot_of_unit[order] = slot_of_sorted
    slot_pix = np.zeros(S, np.int64)
    slot_g = np.zeros(S, f)
    slot_pix[slot_of_unit] = upix
    slot_g[slot_of_unit] = ug
    tile_e = np.asarray(tile_e)

    xf = np.asarray(x, f).reshape(B, IN_C, -1).transpose(0, 2, 1).reshape(NPIX, IN_C)
    xs = xf[slot_pix]

    W1t = np.transpose(np.asarray(W1, f), (0, 2, 1))
    W2t = np.transpose(np.asarray(W2, f), (0, 2, 1))
    W3t = np.transpose(np.asarray(W3, f), (0, 2, 1))
    b1a, b2a, b3a = np.asarray(b1, f), np.asarray(b2, f), np.asarray(b3, f)
    wblk_e = np.zeros((E, 128, WBLK), f)
    for e in range(E):
        wblk_e[e, :, W1_OFF : W1_OFF + HID] = W1t[e, 0:128]
        wblk_e[e, 0:64, W1_OFF + HID : W1_OFF + 2 * HID] = W1t[e, 128:192]
        wblk_e[e, 64, W1_OFF + HID : W1_OFF + 2 * HID] = b1a[e]
        for k in range(3):
            wblk_e[e, :, W2_OFF + k * HID : W2_OFF + (k + 1) * HID] = (
                W2t[e, 128 * k : 128 * (k + 1)]
            )
            wblk_e[e, :, W3_OFF + k * OUT_C : W3_OFF + (k + 1) * OUT_C] = (
                W3t[e, 128 * k : 128 * (k + 1)]
            )
    b2col_e = np.stack([b2a[e].reshape(3, 128).T for e in range(E)])

    # deal PAIRS round-robin to cores
    core_pairs = [np.arange(c, n_pairs, N_CORES) for c in range(N_CORES)]
    in_maps = []
    for c in range(N_CORES):
        xt = np.zeros((NT, 128, 2, TILE), f)
        wb = np.zeros((NP_, 128, WBLK), f)
        gt = np.zeros((NT, 1, TILE + OUT_C), f)
        b2t = np.zeros((NP_, 128, 3), f)
        for j, gp in enumerate(core_pairs[c]):
            e = int(tile_e[2 * gp])
            wb[j] = wblk_e[e]
            b2t[j] = b2col_e[e]
            for h in range(2):
                tg = 2 * gp + h
                tl = 2 * j + h
                sl = slice(tg * TILE, (tg + 1) * TILE)
                xcols = xs[sl].T
                xt[tl, :, 0, :] = xcols[0:128]
                xt[tl, 0:64, 1, :] = xcols[128:192]
                xt[tl, 64, 1, :] = 1.0
                gt[tl, 0, 0:TILE] = slot_g[sl]
                gt[tl, 0, TILE : TILE + OUT_C] = b3a[e]
        in_maps.append(
            {"xt": xt, "wb": wb, "gt": gt, "b2t": b2t, "ones": np.ones((1, 128), f)}
        )
    return in_maps, core_pairs, slot_of_unit, n_tiles


def kernel(x, router_input, router_W, router_b, W1, b1, W2, b2, W3, b3, **run_kwargs):
    nc = _get_nc()
    in_maps, core_pairs, slot_of_unit, n_tiles = make_in_maps(
        x, router_input, router_W, router_b, W1, b1, W2, b2, W3, b3
    )
    res = run_bass_kernel_spmd(nc, in_maps, list(range(N_CORES)), **run_kwargs)

    y_slots = np.zeros((n_tiles, TILE, OUT_C), np.float32)
    for c in range(N_CORES):
        gp = core_pairs[c]
        oc = res.results[c]["out"]                            # [NT, OUT_C, TILE]
        tgs = np.stack([2 * gp, 2 * gp + 1], 1).reshape(-1)
        y_slots[tgs] = oc[: len(tgs)].transpose(0, 2, 1)
    y_slots = y_slots.reshape(n_tiles * TILE, OUT_C)
    out = y_slots[slot_of_unit[0::2]] + y_slots[slot_of_unit[1::2]]
    full = out.reshape(B, H * W, OUT_C).transpose(0, 2, 1).reshape(B, OUT_C, H, W)
    if run_kwargs:
        kernel.last_results = res
    return full


# revision 3
# speedup vs baseline: 1.1877x; 1.1877x over previous
"""MoE update-MLP Trainium2 kernel v4: host-routed top-2 sparsity, 8-core SPMD,
software-pipelined across tiles.

Like v3 (host evaluates the router with the reference's own jax fp32 einsum,
groups the 131072 active (pixel, expert) units by expert into 512-slot tiles,
cores each run NT fixed tile slots, output unsharded host-side by summing each
pixel's two slot contributions), with two throughput fixes:

1. Tile-level software pipeline. v3's per-tile chain L1 -> gelu -> L2 -> gelu
   -> gate-mult -> L3 stalled the PE ~35% of the time waiting on ACT/DVE,
   which also dropped the PE out of its ramped p-state (half-rate ~50% of the
   run). v4 emits per iteration: A(t) = gate-broadcast + L1 matmuls,
   C(t-1) = L3 + b3 + output copy, B(t) = L2 matmuls + gelu + gate-mult, so
   every ACT/DVE stage runs under another tile's matmuls.

2. Tiles are paired by expert (each expert's tile count padded to even, pairs
   dealt round-robin to cores), so one weight-block DMA serves two tiles,
   halving the dominant DMA stream.

Device math is fp32r (full PE rate at free-dim 512, and unlike bf16/fp8 it
avoids the heavy power-throttle observed on dense 16/8-bit streams); PSUM is
fp32; gates stay fp32 end-to-end. L1's bias rides a ones-row in the padded x
chunk; L2 uses per-m ACT bias; b3 enters as a K=1 matmul of the b3 row
against the gate row.
"""

import numpy as np

import concourse.bacc as bacc
import concourse.mybir as mybir
import concourse.tile as tile
from concourse.bass import broadcast_tensor_aps
from concourse.bass_utils import run_bass_kernel_spmd

F32 = mybir.dt.float32
F32R = mybir.dt.float32r
AF = mybir.ActivationFunctionType
ALU = mybir.AluOpType

N_CORES = 8
B, IN_C, H, W = 4, 192, 128, 128
R_C, E, HID, OUT_C = 8, 8, 384, 192
TILE = 512
NP_ = 17                      # weight pairs per core
NT = 2 * NP_                  # tile slots per core
NPIX = B * H * W

W1_OFF = 0
W2_OFF = 2 * HID
W3_OFF = 2 * HID + 3 * HID
WBLK = 2 * HID + 3 * HID + 3 * OUT_C

_nc_cache: dict = {}


def _build(compile: bool = True):
    nc = bacc.Bacc("TRN2", target_bir_lowering=False, debug=False)

    xt_in = nc.declare_dram_parameter("xt", [NT, 128, 2, TILE], F32R, isOutput=False)
    wb_in = nc.declare_dram_parameter("wb", [NP_, 128, WBLK], F32R, isOutput=False)
    gt_in = nc.declare_dram_parameter("gt", [NT, 1, TILE + OUT_C], F32R, isOutput=False)
    b2_in = nc.declare_dram_parameter("b2t", [NP_, 128, 3], F32, isOutput=False)
    ones_in = nc.declare_dram_parameter("ones", [1, 128], F32R, isOutput=False)
    out_d = nc.declare_dram_parameter("out", [NT, OUT_C, TILE], F32, isOutput=True)

    with tile.TileContext(nc) as tc:
        with (
            tc.tile_pool(name="cpool", bufs=1) as cpool,
            tc.tile_pool(name="wpool", bufs=2) as wpool,
            tc.tile_pool(name="xpool", bufs=2) as xpool,
            tc.tile_pool(name="hpool", bufs=2) as hpool,
            tc.tile_pool(name="psL1", bufs=2, space="PSUM") as psL1,
            tc.tile_pool(name="psL2", bufs=1, space="PSUM") as psL2,
            tc.tile_pool(name="psG", bufs=3, space="PSUM") as psG,
        ):
            ones_sb = cpool.tile([1, 128], F32R)
            nc.sync.dma_start(ones_sb[:], ones_in[:])

            w_sb = {}
            x_sb = {}
            g_sb = {}
            b2_sb = {}
            h2g = {}
            o_ps = {}
            o_rd = {}

            def dma_tile(t):
                if t >= NT:
                    return
                if t % 2 == 0:
                    p = t // 2
                    w_sb[p] = wpool.tile([128, WBLK], F32R, tag="w", name=f"w_{p}")
                    nc.gpsimd.dma_start(w_sb[p][:], wb_in[p])
                    b2_sb[p] = xpool.tile([128, 3], F32, tag="b2", name=f"b2_{p}")
                    nc.sync.dma_start(b2_sb[p][:], b2_in[p])
                x_sb[t] = xpool.tile([128, 2, TILE], F32R, tag="x", name=f"x_{t}")
                nc.sync.dma_start(x_sb[t][:], xt_in[t])
                g_sb[t] = xpool.tile(
                    [1, TILE + OUT_C], F32R, tag="g", bufs=3, name=f"g_{t}"
                )
                nc.sync.dma_start(g_sb[t][:], gt_in[t])

            def emit_A(t):
                w = w_sb[t // 2]
                gb_ps = psG.tile([128, TILE], F32, tag="ps_g", name=f"gbps_{t}")
                nc.tensor.matmul(
                    gb_ps[:], ones_sb[:], g_sb[t][:, 0:TILE], start=True, stop=True
                )
                gb = hpool.tile([128, 1, TILE], F32R, tag="gb", name=f"gb_{t}")
                nc.vector.tensor_copy(gb[:, 0, :], gb_ps[:])
                h1 = hpool.tile([128, 3, TILE], F32R, tag="h1", name=f"h1_{t}")
                for m in range(3):
                    ps1 = psL1.tile([128, TILE], F32, tag="ps1", name=f"ps1_{t}_{m}")
                    for c in range(2):
                        col = W1_OFF + c * HID + 128 * m
                        nc.tensor.matmul(
                            ps1[:],
                            w[:, col : col + 128],
                            x_sb[t][:, c, :],
                            start=(c == 0),
                            stop=(c == 1),
                        )
                    nc.scalar.activation(h1[:, m, :], ps1[:], AF.Gelu)
                return h1, gb

            def emit_B(t, h1, gb):
                w = w_sb[t // 2]
                ps2 = psL2.tile([128, 3, TILE], F32, tag="ps2", name=f"ps2_{t}")
                for m in range(3):
                    for k in range(3):
                        col = W2_OFF + k * HID + 128 * m
                        nc.tensor.matmul(
                            ps2[:, m, :],
                            w[:, col : col + 128],
                            h1[:, k, :],
                            start=(k == 0),
                            stop=(k == 2),
                        )
                h2 = hpool.tile([128, 3, TILE], F32R, tag="h2", name=f"h2_{t}")
                h2g[t] = hpool.tile([128, 3, TILE], F32R, tag="h2g", name=f"h2g_{t}")
                for m in range(3):
                    nc.scalar.activation(
                        h2[:, m, :], ps2[:, m, :], AF.Gelu,
                        bias=b2_sb[t // 2][:, m : m + 1],
                    )
                    nc.vector.tensor_mul(h2g[t][:, m, :], h2[:, m, :], gb[:, 0, :])

            def emit_C(t):
                w = w_sb[t // 2]
                o_ps[t] = [
                    psG.tile([128, TILE], F32, tag="ps_g", name=f"o0_{t}"),
                    psG.tile([128, TILE], F32, tag="ps_g", name=f"o1_{t}"),
                ]
                for m, rows in ((0, 128), (1, OUT_C - 128)):
                    op = o_ps[t][m]
                    for k in range(3):
                        col = W3_OFF + k * OUT_C + 128 * m
                        nc.tensor.matmul(
                            op[:rows],
                            w[:, col : col + rows],
                            h2g[t][:, k, :],
                            start=(k == 0),
                            stop=False,
                        )
                    nc.tensor.matmul(
                        op[:rows],
                        g_sb[t][:, TILE + 128 * m : TILE + 128 * m + rows],
                        g_sb[t][:, 0:TILE],
                        start=False,
                        stop=True,
                    )

            def emit_C_read(t):
                for m, rows in ((0, 128), (1, OUT_C - 128)):
                    o_sb = hpool.tile([128, TILE], F32, tag="o_sb", bufs=4)
                    nc.scalar.copy(o_sb[:rows], o_ps[t][m][:rows])
                    nc.sync.dma_start(
                        out_d[t, 128 * m : 128 * m + rows, :], o_sb[:rows]
                    )

            dma_tile(0)
            prev = None
            for t in range(NT):
                dma_tile(t + 1)
                h1, gb = emit_A(t)
                if prev is not None:
                    emit_C(prev)
                    emit_C_read(prev)
                emit_B(t, h1, gb)
                prev = t
            emit_C(prev)
            emit_C_read(prev)

    if compile:
        nc.compile()
    return nc


def _get_nc():
    if "v4" not in _nc_cache:
        _nc_cache["v4"] = _build()
    return _nc_cache["v4"]


def _route(router_input, router_W, router_b):
    """Host router. Logits via the same jax fp32 einsum as the reference so
    the top-2 ranking matches bit-for-bit; gates in fp64 from those logits."""
    import jax
    import jax.numpy as jnp

    cpu = jax.devices("cpu")[0]
    with jax.default_device(cpu):
        logits = np.asarray(
            jnp.einsum(
                "ec,bchw->behw",
                jnp.asarray(router_W, jnp.float32),
                jnp.asarray(router_input, jnp.float32),
            )
            + jnp.asarray(router_b, jnp.float32)[None, :, None, None]
        )
    lt = logits.transpose(0, 2, 3, 1).reshape(-1, E).astype(np.float64)
    top2 = np.argsort(-lt, axis=-1, kind="stable")[:, :2]
    l1 = np.take_along_axis(lt, top2[:, 0:1], 1)[:, 0]
    l2 = np.take_along_axis(lt, top2[:, 1:2], 1)[:, 0]
    e2 = np.exp(l2 - l1)
    g1 = 1.0 / (1.0 + e2)
    g2 = e2 / (1.0 + e2)
    return top2, np.stack([g1, g2], 1).astype(np.float32)


def make_in_maps(x, router_input, router_W, router_b, W1, b1, W2, b2, W3, b3):
    f = np.float32

    top2, gg = _route(router_input, router_W, router_b)

    ue = top2.reshape(-1)
    upix = np.repeat(np.arange(NPIX), 2)
    ug = gg.reshape(-1)
    order = np.argsort(ue, kind="stable")
    counts = np.bincount(ue, minlength=E)

    # pad each expert's tile count to EVEN so pairs share one weight block
    seg_tiles = [int(2 * np.ceil(np.ceil(c / TILE) / 2)) for c in counts]
    n_tiles = sum(seg_tiles)
    n_pairs = n_tiles // 2
    assert n_pairs <= NP_ * N_CORES, (counts, n_tiles)
    S = n_tiles * TILE
    slot_of_sorted = np.zeros(2 * NPIX, np.int64)
    pos = 0
    off = 0
    tile_e = []
    for e in range(E):
        c = int(counts[e])
        slot_of_sorted[off : off + c] = pos + np.arange(c)
        tile_e += [e] * seg_tiles[e]
        pos += seg_tiles[e] * TILE
        off += c
    slot_of_unit = np.zeros(2 * NPIX, np.int64)
    slot_of_unit[order] = slot_of_sorted
    slot_pix = np.zeros(S, np.int64)
    slot_g = np.zeros(S, f)
    slot_pix[slot_of_unit] = upix
    slot_g[slot_of_unit] = ug
    tile_e = np.asarray(tile_e)

    xf = np.asarray(x, f).reshape(B, IN_C, -1).transpose(0, 2, 1).reshape(NPIX, IN_C)
    xs = xf[slot_pix]

    W1t = np.transpose(np.asarray(W1, f), (0, 2, 1))
    W2t = np.transpose(np.asarray(W2, f), (0, 2, 1))
    W3t = np.transpose(np.asarray(W3, f), (0, 2, 1))
    b1a, b2a, b3a = np.asarray(b1, f), np.asarray(b2, f), np.asarray(b3, f)
    wblk_e = np.zeros((E, 128, WBLK), f)
    for e in range(E):
        wblk_e[e, :, W1_OFF : W1_OFF + HID] = W1t[e, 0:128]
        wblk_e[e, 0:64, W1_OFF + HID : W1_OFF + 2 * HID] = W1t[e, 128:192]
        wblk_e[e, 64, W1_OFF + HID : W1_OFF + 2 * HID] = b1a[e]
        for k in range(3):
            wblk_e[e, :, W2_OFF + k * HID : W2_OFF + (k + 1) * HID] = (
                W2t[e, 128 * k : 128 * (k + 1)]
            )
            wblk_e[e, :, W3_OFF + k * OUT_C : W3_OFF + (k + 1) * OUT_C] = (
                W3t[e, 128 * k : 128 * (k + 1)]
            )
    b2col_e = np.stack([b2a[e].reshape(3, 128).T for e in range(E)])

    # deal PAIRS round-robin to cores
    core_pairs = [np.arange(c, n_pairs, N_CORES) for c in range(N_CORES)]
    in_maps = []
    for c in range(N_CORES):
        xt = np.zeros((NT, 128, 2, TILE), f)
        wb = np.zeros((NP_, 128, WBLK), f)
        gt = np.zeros((NT, 1, TILE + OUT_C), f)
        b2t = np.zeros((NP_, 128, 3), f)
        for j, gp in enumerate(core_pairs[c]):
            e = int(tile_e[2 * gp])
            wb[j] = wblk_e[e]
            b2t[j] = b2col_e[e]
            for h in range(2):
                tg = 2 * gp + h
                tl = 2 * j + h
                sl = slice(tg * TILE, (tg + 1) * TILE)
                xcols = xs[sl].T
                xt[tl, :, 0, :] = xcols[0:128]
                xt[tl, 0:64, 1, :] = xcols[128:192]
                xt[tl, 64, 1, :] = 1.0
                gt[tl, 0, 0:TILE] = slot_g[sl]
                gt[tl, 0, TILE : TILE + OUT_C] = b3a[e]
        in_maps.append(
            {"xt": xt, "wb": wb, "gt": gt, "b2t": b2t, "ones": np.ones((1, 128), f)}
        )
    return in_maps, core_pairs, slot_of_unit, n_tiles


def kernel(x, router_input, router_W, router_b, W1, b1, W2, b2, W3, b3, **run_kwargs):
    nc = _get_nc()
    in_maps, core_pairs, slot_of_unit, n_tiles = make_in_maps(
        x, router_input, router_W, router_b, W1, b1, W2, b2, W3, b3
    )
    res = run_bass_kernel_spmd(nc, in_maps, list(range(N_CORES)), **run_kwargs)

    y_slots = np.zeros((n_tiles, TILE, OUT_C), np.float32)
    for c in range(N_CORES):
        gp = core_pairs[c]
        oc = res.results[c]["out"]                            # [NT, OUT_C, TILE]
        tgs = np.stack([2 * gp, 2 * gp + 1], 1).reshape(-1)
        y_slots[tgs] = oc[: len(tgs)].transpose(0, 2, 1)
    y_slots = y_slots.reshape(n_tiles * TILE, OUT_C)
    out = y_slots[slot_of_unit[0::2]] + y_slots[slot_of_unit[1::2]]
    full = out.reshape(B, H * W, OUT_C).transpose(0, 2, 1).reshape(B, OUT_C, H, W)
    if run_kwargs:
        kernel.last_results = res
    return full
